# revision 36
# baseline (speedup 1.0000x reference)
"""Trainium2 Bass kernel for causal multi-head attention (v2).

Problem: B=2, S=2048, HID=2048, H=16 heads, DH=128, causal mask.
  Q = X @ Wq.T ; K = X @ Wk.T ; V = X @ Wv.T          (per-head split)
  out = softmax(mask(Q K^T / sqrt(DH))) V  @ Wo.T + bo

Sharding over 8 cores: core c = (b, g) with b = c // 4 (batch),
g = c % 4 (head group of 4 heads = 512 hidden dims).  Each core computes
its batch's full attention for its 4 heads plus a partial output
projection (its 512 input dims of Wo); the host sums the 4 partials per
batch, rescales, and adds the bias.

v2 speedup over the fp32r v1: the two long-contraction GEMM families
(QKV projections, K=2048; output projection, K=512) run as
*compensated fp8* DoubleRow matmuls.  Each operand X is split on the
host (or on-device for y) into X ~ Xhi + Xlo with both parts quantized
to fp8e4m3 at a common scale; the product is computed as three
DoubleRow chains  Whi@Xhi + Wlo@Xhi + Whi@Xlo  accumulating in the same
PSUM region.  DoubleRow processes two 128-deep k-tiles per instruction
at 0.5 PE cycles per output row, so the three products cost 12 rows per
16 fp32r rows -> 1.33x on those stages at ~bf16 accuracy (the dropped
Wlo@Xlo term is O(2^-8)).  The attention core (scores, PV, row-sum)
contracts over only 128/256 elements where compensation cannot beat
bf16, so it runs in plain bf16 at 1 cycle/row with exact causal tile
widths (bf16 has no fp32r 256-wide floor, so diagonal blocks shrink to
128).

Engine split: ScalarE runs exp exclusively in phase 2 and the Q/K PSUM
evacuations (Copy with 1/128 scale) in phase 1; DVE handles V/yt
evacuations, causal mask multiplies (bf16 in SBUF -> 4x DVE mode), and
reciprocals; Pool (gpsimd) takes the output-projection PSUM
evacuations.  Softmax skips max-subtraction (scores bounded ~+-6; exp
gets a constant -2 bias so everything stays comfortably in range).
Scores are computed transposed (S^T[k, q]) so probability tiles feed PV
directly as the moving operand with V as lhsT - no on-device
transposes.  The row-sum over keys is a ones-vector matmul broadcast to
all partitions; division is one reciprocal + multiply on DVE.

The attention inner loop keeps v1's software pipelining: score matmuls
for iteration i+1 are emitted before the PV/normalization matmuls of
iteration i so the PE never waits on the scalar engine's exp stream.
"""

import sys

sys.path.insert(0, "/opt/trn_rl_repo")

from collections import deque
from contextlib import ExitStack

import ml_dtypes
import numpy as np

import concourse.bass as bass  # noqa: F401
import concourse.tile as tile
from concourse import bacc, mybir
from concourse.bass_utils import run_bass_kernel_spmd

B = 2
S = 2048
HID = 2048
H = 16
DH = 128
SCALE = np.float32(1.0 / np.sqrt(DH))
RS = np.float32(np.sqrt(SCALE))  # sqrt of softmax scale, folded into Wq AND Wk

N_CORES = 8
HPC = 4  # heads per core
JG = HPC * DH  # 512: hidden dims per core's head group
P = 128
NT = 512  # matmul free-dim tile (= 1 PSUM bank of fp32)
KT = HID // P  # 16 contraction tiles for the QKV projections
KP = KT // 2  # 8 DoubleRow k-tile pairs
SB = S // NT  # 4 seq blocks of 512
QB = S // P  # 16 key blocks of 128

WQSC = np.float32(128.0)  # fp8 scale for Wq/Wk (* RS)
WVSC = np.float32(64.0)  # fp8 scale for Wv
WOSC = np.float32(64.0)  # fp8 scale for Wo (undone on host)

f32 = mybir.dt.float32
bf16 = mybir.dt.bfloat16
fp8 = mybir.dt.float8e4
Exp = mybir.ActivationFunctionType.Exp
Copy = mybir.ActivationFunctionType.Copy
DR = mybir.MatmulPerfMode.DoubleRow

E4NP = ml_dtypes.float8_e4m3
BFNP = ml_dtypes.bfloat16

_COMPILED = None
AB_BF16_RS = False


def _emit(nc, tc):
    xhi_d = nc.dram_tensor("XHI", [HID, S], fp8, kind="ExternalInput").ap()
    xlo_d = nc.dram_tensor("XLO", [HID, S], fp8, kind="ExternalInput").ap()
    wqh_d = nc.dram_tensor("WQH", [HID, JG], fp8, kind="ExternalInput").ap()
    wql_d = nc.dram_tensor("WQL", [HID, JG], fp8, kind="ExternalInput").ap()
    wkh_d = nc.dram_tensor("WKH", [HID, JG], fp8, kind="ExternalInput").ap()
    wkl_d = nc.dram_tensor("WKL", [HID, JG], fp8, kind="ExternalInput").ap()
    wvh_d = nc.dram_tensor("WVH", [HID, JG], fp8, kind="ExternalInput").ap()
    wvl_d = nc.dram_tensor("WVL", [HID, JG], fp8, kind="ExternalInput").ap()
    woh_d = nc.dram_tensor("WOH", [JG, HID], fp8, kind="ExternalInput").ap()
    wol_d = nc.dram_tensor("WOL", [JG, HID], fp8, kind="ExternalInput").ap()
    mb_d = nc.dram_tensor("MB", [P, 896], bf16, kind="ExternalInput").ap()
    ones_d = nc.dram_tensor("ONES", [P, P], bf16, kind="ExternalInput").ap()
    ot_d = nc.dram_tensor("OT", [HID, S], bf16, kind="ExternalOutput").ap()

    with ExitStack() as top:
        dpool = top.enter_context(tc.tile_pool(name="dram", bufs=1, space="DRAM"))
        qt_dram = dpool.tile([JG, S], bf16)
        kt_dram = dpool.tile([JG, S], bf16)

        # Long-lived SBUF: V stays resident from phase 1 through phase 2;
        # per-head Q^T/K^T tiles are double-buffered; constants.
        vpool = top.enter_context(tc.tile_pool(name="v", bufs=1))
        hpool = top.enter_context(tc.tile_pool(name="h", bufs=4))
        cpool = top.enter_context(tc.tile_pool(name="c", bufs=1))
        v_sb = vpool.tile([P, QB, JG], bf16)
        mb_sb = cpool.tile([P, 896], bf16)
        ones_sq = cpool.tile([P, P], bf16)
        # fp8 all-ones pair for the DoubleRow row-sum over off-diagonal
        # probability pairs (1.0 is exact in fp8e4m3)
        ones8 = cpool.tile([P, 2, P], fp8)
        nc.gpsimd.memset(ones8[:], 1.0)
        # per-partition scalar bias for exp(s - 2): keeps bf16 probs small
        bias_m2 = cpool.tile([P, 1], f32)
        nc.gpsimd.memset(bias_m2[:], -2.0)

        # First two heads' Q^T/K^T tiles: column chunks are loaded
        # mid-phase-1 (once the DMA inflow crunch is over) so attention can
        # start the moment the projections finish.
        qkt = {}
        for h in range(HPC):
            qt_h = hpool.tile([P, S], bf16, name=f"qt{h}", tag="qt")
            kt_h = hpool.tile([P, S], bf16, name=f"kt{h}", tag="kt")
            qkt[h] = (qt_h, kt_h)

        # Output-projection weights and yt hi/lo live across phases 2+3;
        # the wo loads are issued mid-phase-1 so they never contend with the
        # phase-boundary staging DMAs.
        wopool = top.enter_context(tc.tile_pool(name="wo", bufs=1))
        ypool = top.enter_context(tc.tile_pool(name="y", bufs=1))
        woh_sb = wopool.tile([P, HPC, HID], fp8)
        wol_sb = wopool.tile([P, HPC, HID], fp8)
        yth_sb = ypool.tile([P, HPC, S], fp8)
        ytl_sb = ypool.tile([P, HPC, S], fp8)

        # ------------------- Phase 1: QKV projections -------------------
        with ExitStack() as p1:
            wpool = p1.enter_context(tc.tile_pool(name="w", bufs=1))
            xpool = p1.enter_context(tc.tile_pool(name="x", bufs=2))
            spool = p1.enter_context(tc.tile_pool(name="s1", bufs=2))
            ppool = p1.enter_context(tc.tile_pool(name="p1", bufs=8, space="PSUM"))

            wqh_sb = wpool.tile([P, KT, JG], fp8)
            wql_sb = wpool.tile([P, KT, JG], fp8)
            wkh_sb = wpool.tile([P, KT, JG], fp8)
            wkl_sb = wpool.tile([P, KT, JG], fp8)
            wvh_sb = wpool.tile([P, KT, JG], fp8)
            wvl_sb = wpool.tile([P, KT, JG], fp8)
            w_aps = {}
            for nm, d in (("qh", wqh_d), ("ql", wql_d), ("kh", wkh_d),
                          ("kl", wkl_d), ("vh", wvh_d), ("vl", wvl_d)):
                w_aps[nm] = d.rearrange("(kt p) j -> p kt j", p=P)
            w_sbs = {"qh": wqh_sb, "ql": wql_sb, "kh": wkh_sb,
                     "kl": wkl_sb, "vh": wvh_sb, "vl": wvl_sb}

            # DMA issue order follows compute demand: sb0's X first, then the
            # Q weights, K weights, V weights, then constants.
            def load_x(sb):
                xh = xpool.tile([P, KT, NT], fp8, name=f"xh{sb}", tag="xh")
                xl = xpool.tile([P, KT, NT], fp8, name=f"xl{sb}", tag="xl")
                csl = slice(sb * NT, (sb + 1) * NT)
                nc.sync.dma_start(
                    xh[:], xhi_d.rearrange("(kt p) s -> p kt s", p=P)[:, :, csl])
                nc.sync.dma_start(
                    xl[:], xlo_d.rearrange("(kt p) s -> p kt s", p=P)[:, :, csl])
                return xh, xl

            # First seq block's X and the Q weights arrive chunked and
            # interleaved so the first product chain starts ~2.5us in
            # instead of waiting for four full-tensor DMAs (~12us).
            xh0 = xpool.tile([P, KT, NT], fp8, name="xh0", tag="xh")
            xl0 = xpool.tile([P, KT, NT], fp8, name="xl0", tag="xl")
            xh_ap = xhi_d.rearrange("(kt p) s -> p kt s", p=P)
            xl_ap = xlo_d.rearrange("(kt p) s -> p kt s", p=P)
            # first X chunk rides the otherwise-idle Pool SWDGE path so its
            # descriptor generation overlaps the W chunks' shared HWDGE queue
            nc.gpsimd.dma_start(xh0[:, 0:4], xh_ap[:, 0:4, 0:NT])
            for c in range(4):
                ksl = slice(4 * c, 4 * c + 4)
                nc.sync.dma_start(wqh_sb[:, ksl], w_aps["qh"][:, ksl])
                if c > 0:
                    nc.sync.dma_start(xh0[:, ksl], xh_ap[:, ksl, 0:NT])
            nc.gpsimd.dma_start(xl0[:, 0:4], xl_ap[:, 0:4, 0:NT])
            for c in range(4):
                ksl = slice(4 * c, 4 * c + 4)
                nc.sync.dma_start(wql_sb[:, ksl], w_aps["ql"][:, ksl])
                if c > 0:
                    nc.sync.dma_start(xl0[:, ksl], xl_ap[:, ksl, 0:NT])
            xts0 = (xh0, xl0)
            for nm in ("kh", "kl"):
                for c in range(2):
                    ksl = slice(8 * c, 8 * c + 8)
                    nc.sync.dma_start(w_sbs[nm][:, ksl], w_aps[nm][:, ksl])
            for nm in ("vh", "vl"):
                nc.sync.dma_start(w_sbs[nm][:], w_aps[nm])
            nc.sync.dma_start(woh_sb[:],
                              woh_d.rearrange("(kt p) o -> p kt o", p=P))
            nc.sync.dma_start(wol_sb[:],
                              wol_d.rearrange("(kt p) o -> p kt o", p=P))
            nc.sync.dma_start(mb_sb[:], mb_d[:])
            nc.sync.dma_start(ones_sq[:], ones_d[:])
            # pre-warm ScalarE's Exp table set so the first attention exp
            # doesn't pay the table load mid-pipeline
            warm = cpool.tile([1, 1], f32)
            nc.scalar.activation(warm[:], mb_sb[0:1, 0:1], Exp)

            def qk_passes(sb, xh, xl):
                # Q^T and K^T: [jg, s] tiles; three compensated DoubleRow
                # product chains accumulate into one PSUM bank per m.
                for pname, wh_sb, wl_sb, dst, sc in (
                        ("q", wqh_sb, wql_sb, qt_dram, 1.0 / WQSC),
                        ("k", wkh_sb, wkl_sb, kt_dram, 1.0 / WQSC)):
                    pts = [ppool.tile([P, NT], f32, name=f"pp{sb}_{m}",
                                      tag="pp") for m in range(HPC)]
                    for m in range(HPC):
                        msl = slice(m * P, (m + 1) * P)
                        for t in range(KP):
                            tsl = slice(2 * t, 2 * t + 2)
                            nc.tensor.matmul(
                                pts[m][:], wh_sb[:, tsl, msl], xh[:, tsl, :],
                                start=(t == 0), stop=False, perf_mode=DR)
                        for t in range(KP):
                            tsl = slice(2 * t, 2 * t + 2)
                            nc.tensor.matmul(
                                pts[m][:], wl_sb[:, tsl, msl], xh[:, tsl, :],
                                start=False, stop=False, perf_mode=DR)
                        for t in range(KP):
                            tsl = slice(2 * t, 2 * t + 2)
                            nc.tensor.matmul(
                                pts[m][:], wh_sb[:, tsl, msl], xl[:, tsl, :],
                                start=False, stop=(t == KP - 1), perf_mode=DR)
                    st = spool.tile([P, HPC, NT], bf16,
                                    name=f"st{sb}_{pname}", tag="st")
                    for m in range(HPC):
                        nc.scalar.activation(st[:, m], pts[m][:], Copy,
                                             scale=float(sc))
                    nc.sync.dma_start(
                        dst[:].rearrange("(m p) s -> p m s", p=P)[
                            :, :, sb * NT:(sb + 1) * NT],
                        st[:])

            def v_pass(sb, xh, xl):
                # V natural layout [s, jg] accumulates straight into SBUF;
                # X is the (compensated) stationary side here.
                pts = [ppool.tile([P, NT], f32, name=f"ppv{sb}_{m}",
                                  tag="pp") for m in range(HPC)]
                for m in range(HPC):
                    msl = slice(m * P, (m + 1) * P)
                    for t in range(KP):
                        tsl = slice(2 * t, 2 * t + 2)
                        nc.tensor.matmul(
                            pts[m][:], xh[:, tsl, msl], wvh_sb[:, tsl, :],
                            start=(t == 0), stop=False, perf_mode=DR)
                    for t in range(KP):
                        tsl = slice(2 * t, 2 * t + 2)
                        nc.tensor.matmul(
                            pts[m][:], xl[:, tsl, msl], wvh_sb[:, tsl, :],
                            start=False, stop=False, perf_mode=DR)
                    for t in range(KP):
                        tsl = slice(2 * t, 2 * t + 2)
                        nc.tensor.matmul(
                            pts[m][:], xh[:, tsl, msl], wvl_sb[:, tsl, :],
                            start=False, stop=(t == KP - 1), perf_mode=DR)
                for m in range(HPC):
                    nc.vector.tensor_scalar_mul(
                        v_sb[:, sb * HPC + m, :], pts[m][:],
                        float(1.0 / WVSC))

            for sb in range(SB):
                xh, xl = xts0 if sb == 0 else load_x(sb)

                # sb0 follows the weight-arrival order (Q, K then V); later
                # blocks run V first so the phase boundary ends on Q/K whose
                # ScalarE evacuations free their PSUM banks quickly, letting
                # the first attention scores start without waiting on DVE.
                if sb == 0:
                    qk_passes(sb, xh, xl)
                    v_pass(sb, xh, xl)
                else:
                    v_pass(sb, xh, xl)
                    qk_passes(sb, xh, xl)

                # hoisted head-0/1 chunk loads, emitted only after the
                # inflow-bound first seq block has cleared the DMA engines
                if sb in (1, 2):
                    for h in range(HPC):
                        qt_h, kt_h = qkt[h]
                        hsl = slice(h * P, (h + 1) * P)
                        for n in range(2) if sb == 1 else (2,):
                            csl = slice(n * NT, (n + 1) * NT)
                            nc.sync.dma_start(kt_h[:, csl], kt_dram[hsl, csl])
                            nc.sync.dma_start(qt_h[:, csl], qt_dram[hsl, csl])

        # ---------------- Phases 2+3 ---------------
        if True:
            # -------------- Phases 2+3: attention + projection ------------
            # A "pending" queue of deferred PE work (PV + row-sum matmuls of
            # the previous attention iteration, and output-projection column
            # blocks once the last head finishes a column) is drained between
            # score matmuls so the PE never waits on ScalarE's exp stream.
            with ExitStack() as p2:
                epool = p2.enter_context(tc.tile_pool(name="e", bufs=36))
                mpool = p2.enter_context(tc.tile_pool(name="m", bufs=2))
                s3pool = p2.enter_context(tc.tile_pool(name="s3", bufs=2))
                pspool = p2.enter_context(
                    tc.tile_pool(name="p2", bufs=1, space="PSUM"))

                items = [(h, qb) for qb in range(SB) for h in range(HPC)]
                state = {}
                pending = deque()

                def drain(n):
                    for _ in range(min(n, len(pending))):
                        pending.popleft()()

                def emit_a(it):
                    h, qb = items[it]
                    if qb == 0:
                        # chunks 0-2 were hoisted into phase 1; chunk 3 is
                        # only produced by the last seq block, load it here
                        qt_h, kt_h = qkt[h]
                        csl = slice(3 * NT, 4 * NT)
                        nc.sync.dma_start(
                            kt_h[:, csl], kt_dram[h * P:(h + 1) * P, csl])
                        nc.sync.dma_start(
                            qt_h[:, csl], qt_dram[h * P:(h + 1) * P, csl])
                    qt_h, kt_h = qkt[h]
                    nkb = 4 * qb + 4
                    ets = []
                    et8s = []
                    etp = None
                    for kb in range(nkb):
                        # Diagonal key blocks only need queries q >= k: shrink
                        # the free dim to the exact causal width.
                        r = kb - 4 * qb
                        stq = 0 if r < 0 else 128 * r
                        w = NT - stq
                        ps_s = pspool.tile([P, NT], f32,
                                           name=f"ps{h}_{qb}_{kb}",
                                           tag="ps_s", bufs=3)
                        nc.tensor.matmul(
                            ps_s[:, :w], kt_h[:, kb * P:(kb + 1) * P],
                            qt_h[:, qb * NT + stq:(qb + 1) * NT],
                            start=True, stop=True)
                        if r < 0:
                            # off-diagonal tiles come in pairs sharing one
                            # bf16 tile; Pool converts each completed pair
                            # to fp8 for the DoubleRow row-sum
                            if kb % 2 == 0:
                                etp = epool.tile([P, 2, NT], bf16,
                                                 name=f"etp{h}_{qb}_{kb//2}",
                                                 tag="etp", bufs=14)
                            nc.scalar.activation(etp[:, kb % 2, :], ps_s[:],
                                                 Exp, bias=bias_m2[:])
                            ets.append((etp, kb % 2, stq, w))
                            if kb % 2 == 1:
                                et8 = epool.tile([P, 2, NT], fp8,
                                                 name=f"et8{h}_{qb}_{kb//2}",
                                                 tag="et8", bufs=14)
                                # alternate the fp8 conversion between the
                                # otherwise-idle Pool and DVE: one engine
                                # alone cannot keep up with 6 pairs/item
                                import os as _os
                                _c8 = _os.environ.get("ET8_ENG", "mix")
                                if _c8 == "pool":
                                    ceng = nc.gpsimd
                                elif _c8 == "dve":
                                    ceng = nc.vector
                                else:
                                    ceng = nc.gpsimd if (kb // 2) % 2 == 0 \
                                        else nc.vector
                                ceng.tensor_copy(et8[:], etp[:])
                                et8s.append((etp, et8))
                        else:
                            et = epool.tile([P, NT], bf16,
                                            name=f"et{h}_{qb}_{kb}", tag="et",
                                            bufs=10)
                            nc.scalar.activation(et[:, :w], ps_s[:, :w], Exp,
                                                 bias=bias_m2[:])
                            # diagonal block: causal mask
                            # element [p, f] allowed iff f >= 128*r - stq + p
                            off = 384 - (128 * r - stq)
                            nc.vector.tensor_mul(
                                et[:, :w], et[:, :w],
                                mb_sb[:, off:off + w])
                            ets.append((et, None, stq, w))
                        drain(3)
                    state[it] = (h, qb, nkb, ets, et8s)

                def push_b(it):
                    h, qb, nkb, ets, et8s = state.pop(it)
                    qsl = slice(qb * NT, (qb + 1) * NT)
                    ps_u = pspool.tile([P, NT], f32, name=f"pu{h}_{qb}",
                                       tag="ps_u", bufs=2)
                    ps_rb = pspool.tile([P, NT], f32, name=f"prb{h}_{qb}",
                                        tag="ps_rb", bufs=1)

                    def pv(kb):
                        et, par, stq, w = ets[kb]
                        mv = et[:, :w] if par is None else et[:, par, :]
                        nc.tensor.matmul(
                            ps_u[:, stq:], v_sb[:, kb, h * P:(h + 1) * P],
                            mv, start=(kb == 0), stop=(kb == nkb - 1))

                    # row-sum over keys (partition axis) broadcast to all
                    # partitions: diagonal tiles in bf16, off-diagonal pairs
                    # as one fp8 DoubleRow each (error lands only in the
                    # softmax denominator, ~0.2%)
                    n_rs = 4 + len(et8s)

                    def rs_diag(i, r):
                        et, _, stq, w = ets[4 * qb + r]
                        nc.tensor.matmul(
                            ps_rb[:, stq:], ones_sq[:], et[:, :w],
                            start=(i == 0), stop=(i == n_rs - 1))

                    def rs_pair(i, u):
                        if AB_BF16_RS:  # bf16 rowsum over the pair halves
                            etp_u, _ = et8s[u]
                            nc.tensor.matmul(
                                ps_rb[:], ones_sq[:], etp_u[:, 0, :],
                                start=(i == 0), stop=False)
                            nc.tensor.matmul(
                                ps_rb[:], ones_sq[:], etp_u[:, 1, :],
                                start=False, stop=(i == n_rs - 1))
                        else:
                            nc.tensor.matmul(
                                ps_rb[:], ones8[:], et8s[u][1][:],
                                start=(i == 0), stop=(i == n_rs - 1),
                                perf_mode=DR)

                    rb = mpool.tile([P, NT], f32, name=f"rb{h}_{qb}",
                                    tag="rb", bufs=2)

                    def recip():
                        nc.vector.reciprocal(rb[:], ps_rb[:])

                    def fin():
                        ytf = mpool.tile([P, NT], f32, name=f"ytf{h}_{qb}",
                                         tag="ytf", bufs=2)
                        nc.vector.tensor_mul(ytf[:], ps_u[:], rb[:])
                        nc.vector.tensor_copy(yth_sb[:, h, qsl], ytf[:])
                        nc.vector.tensor_sub(ytl_sb[:, h, qsl], ytf[:],
                                             yth_sb[:, h, qsl])

                    # row-sums first (diagonal r0 leads: full-width start),
                    # then the reciprocal (which overlaps the PV chain on
                    # DVE), then PV, then the yt hi/lo split
                    for r in range(4):
                        pending.append(lambda i=r, r=r: rs_diag(i, r))
                    for u in range(len(et8s)):
                        pending.append(lambda i=4 + u, u=u: rs_pair(i, u))
                    pending.append(recip)
                    for kb in range(nkb):
                        pending.append(lambda kb=kb: pv(kb))
                    pending.append(fin)

                def push_proj_col(n):
                    # output projection for sequence column block n;
                    # requires yt hi+lo [:, :, n*NT:(n+1)*NT] for all heads.
                    nsl = slice(n * NT, (n + 1) * NT)
                    so = s3pool.tile([P, HID // P, NT], bf16,
                                     name=f"so{n}", tag="so")

                    def col_m(m):
                        po = pspool.tile([P, NT], f32, name=f"po{m}_{n}",
                                         tag="po", bufs=2)
                        msl = slice(m * P, (m + 1) * P)
                        # t-major order: the t=0 products contract heads 0-1
                        # only, so they can run while the last head's yt
                        # split is still finishing on DVE
                        for t in range(2):
                            tsl = slice(2 * t, 2 * t + 2)
                            for wsb, ysb in ((woh_sb, yth_sb),
                                             (wol_sb, yth_sb),
                                             (woh_sb, ytl_sb)):
                                first = t == 0 and wsb is woh_sb and ysb is yth_sb
                                last = t == 1 and ysb is ytl_sb
                                nc.tensor.matmul(
                                    po[:], wsb[:, tsl, msl], ysb[:, tsl, nsl],
                                    start=first, stop=last, perf_mode=DR)
                        # alternate evacuation between DVE and ScalarE
                        # (gpsimd cannot read PSUM) so neither becomes the
                        # phase-3 bottleneck
                        if m % 2 == 0:
                            nc.vector.tensor_copy(so[:, m, :], po[:])
                        else:
                            nc.scalar.activation(so[:, m, :], po[:], Copy)
                        # per-m writeback, alternating Pool-SWDGE and
                        # SP-HWDGE queues: the column's output drains as it
                        # is produced, and neither DGE path serializes the
                        # tail (Pool descriptor-gen alone is ~1us per DMA)
                        weng = nc.sync if m % 2 == 0 else nc.gpsimd
                        weng.dma_start(
                            ot_d[m * P:(m + 1) * P, nsl], so[:, m, :])

                    for m in range(HID // P):
                        pending.append(lambda m=m: col_m(m))

                for it in range(len(items)):
                    emit_a(it)
                    if it > 0:
                        push_b(it - 1)
                    h, qb = items[it - 1] if it > 0 else (None, None)
                    if h == HPC - 1:  # last head: this column is complete
                        push_proj_col(qb)
                push_b(len(items) - 1)
                push_proj_col(SB - 1)
                drain(len(pending))


def _build():
    nc = bacc.Bacc("TRN2", target_bir_lowering=False, debug=False,
                   num_devices=N_CORES)
    with tile.TileContext(nc) as tc, \
            nc.allow_low_precision(reason="fp8/bf16 intermediates"):
        _emit(nc, tc)
    nc.compile()
    return nc


def _get_compiled():
    global _COMPILED
    if _COMPILED is None:
        _COMPILED = _build()
    return _COMPILED


def _q8(x):
    return np.clip(x, -240.0, 240.0).astype(E4NP)


def _hi_lo(x):
    hi = _q8(x)
    lo = _q8(x - hi.astype(np.float32))
    return hi, lo


def _make_in_maps(Q_input, Wq, Wk, Wv, Wo):
    mb = (np.arange(896, dtype=np.int32)[None, :] - 384
          >= np.arange(P, dtype=np.int32)[:, None]).astype(BFNP)
    ones = np.ones((P, P), dtype=BFNP)
    in_maps = []
    whl = {}
    for g in range(4):
        gs = slice(g * JG, (g + 1) * JG)
        whl[g] = (
            _hi_lo((Wq[gs, :] * (RS * WQSC)).T),
            _hi_lo((Wk[gs, :] * (RS * WQSC)).T),
            _hi_lo((Wv[gs, :] * WVSC).T),
            _hi_lo((Wo[:, gs] * WOSC).T),
        )
    xhl = {}
    for b in range(B):
        xhl[b] = _hi_lo(np.ascontiguousarray(Q_input[b].T))
    for c in range(N_CORES):
        b, g = divmod(c, 4)
        (wqh, wql), (wkh, wkl), (wvh, wvl), (woh, wol) = whl[g]
        xh, xl = xhl[b]
        in_maps.append({
            "XHI": xh, "XLO": xl,
            "WQH": wqh, "WQL": wql,
            "WKH": wkh, "WKL": wkl,
            "WVH": wvh, "WVL": wvl,
            "WOH": woh, "WOL": wol,
            "MB": mb,
            "ONES": ones,
        })
    return in_maps


def run(Q_input, Wq, Wk, Wv, Wo, bo, trace=False, tmpdir=None):
    nc = _get_compiled()
    in_maps = _make_in_maps(Q_input, Wq, Wk, Wv, Wo)
    last_err = None
    for attempt in range(3):
        try:
            res = run_bass_kernel_spmd(nc, in_maps,
                                       core_ids=list(range(N_CORES)),
                                       trace=trace, tmpdir=tmpdir)
            break
        except Exception as e:  # transient device errors seen on this fabric
            last_err = e
            import time as _time
            _time.sleep(2.0 * (attempt + 1))
    else:
        raise last_err
    out = np.empty((B, S, HID), dtype=np.float32)
    for b in range(B):
        acc = res.results[4 * b]["OT"].astype(np.float32)
        for g in range(1, 4):
            acc += res.results[4 * b + g]["OT"].astype(np.float32)
        out[b] = acc.T * (1.0 / WOSC) + bo[None, :]
    return out, res


def kernel(Q_input, Wq, Wk, Wv, Wo, bo, attention_mask=None, **_ignored):
    Q_input = np.asarray(Q_input, dtype=np.float32)
    Wq = np.asarray(Wq, dtype=np.float32)
    Wk = np.asarray(Wk, dtype=np.float32)
    Wv = np.asarray(Wv, dtype=np.float32)
    Wo = np.asarray(Wo, dtype=np.float32)
    bo = np.asarray(bo, dtype=np.float32)
    out, _ = run(Q_input, Wq, Wk, Wv, Wo, bo, trace=False)
    return out
